# revision 39
# baseline (speedup 1.0000x reference)
"""Trainium2 Bass kernel for the SCON linear-SDE particle scan.

Reference computation: x_{t+1} = (I + DT*W_{t+1}) x_t + DT*b_{t+1} + ds*eps_t
over 10000 steps for B=512 particles with a 3-dim state, observed every 50
steps through a [4,3] projection -> loc_y [512, 201, 4].

The transition matrices depend only on theta (14 scalars), so the whole scan
is a linear map of (x0, eps).  On the host (float64) we precompute propagator
weights that turn the scan into two levels of PE matmuls over the noise:

  level A: each matmul covers 4 chunks of 10 steps; the S50 window suffix-
           products are folded into the weights so the PE accumulates
           window sums U50[w] directly in PSUM (no intermediate U10 level).
           Chunks of the same window sum across K-rows automatically; the
           32-aligned PSUM constraint is met with zero-prefix weight columns.
  level C: obs propagation + projection + x0/deterministic affine part

B is sharded 64 particles per core across 8 cores (pure data parallel).
Per-core device work: stream 3.84 MB of fp16 noise (as lhsT-ready
[128, 250*64] tiles), 260 + 21 matmuls, write [64, 804] fp16 output.
"""

import numpy as np

# ---------------------------------------------------------------- constants
T_TOT = 1000.0
DT = 0.1
N = 10001
TEMP_REF = 283.0
TEMP_RISE = 5.0
GAS_R = 0.008314
NSTEP = N - 1            # 10000
B = 512
NCORE = 8
BC = B // NCORE          # 64 particles per core

L1 = 10                  # level-A chunk length (steps)
NC1 = NSTEP // L1        # 1000 chunks
CPW = 5                  # chunks per window
NW = NC1 // CPW          # 200 windows
NOBS = NW + 1            # 201 observations
OBS_EVERY = 50

SUPER = 4                # chunks per level-A matmul
NMM_A = NC1 // SUPER     # 250 level-A matmuls
KE = 128                 # eps rows per level-A matmul (4 chunks x 32, 2 pad
                         # rows per chunk so blocks start 32-aligned)
NSUP_COL = 250           # eps columns groups (one per matmul)
NTILE_A = (NMM_A + 3) // 4   # 63 psum tiles (4 matmuls/tile, last has 2)

WPS = 10                 # windows per level-B slot (30 rows of 32)
NSLOT_B = NW // WPS      # 20 slots
NTILE_B = NSLOT_B // 4   # 5 u50 tiles
TAUS_PER_SLOT = 4        # u10 tiles touched per level-B slot

NOUT = 4 * NOBS          # 804
NH = NOUT // 2           # 402  (psum free-dim per matmul)

_program_cache = None
_last_results = None     # BassKernelResults of the most recent run (for test.py)

# Validation builds add PSUM memsets so CoreSim's race checker sees no reads
# of never-written rows.  Production skips them: the stale rows only ever
# multiply zero weight columns (and are overwritten data-wise each reuse), so
# they cannot affect results; first-use tiles are still zeroed.
SIM_SAFE = False


# ------------------------------------------------------------- host math
def _forcings():
    times = np.linspace(0.0, T_TOT, N)
    temp = (TEMP_REF + TEMP_RISE * times / (80 * 24 * 365)
            + 10 * np.sin(2 * np.pi / 24 * times)
            + 10 * np.sin(2 * np.pi / (24 * 365) * times))
    I_S = 0.001 + 0.0005 * np.sin(2 * np.pi / (24 * 365) * times)
    I_D = 0.0001 + 5e-05 * np.sin(2 * np.pi / (24 * 365) * times)
    return temp, I_S, I_D


F16 = np.float16


def _a_entries():
    """Level-A matmul entries (theta-independent structure).

    Supergroup s covers chunks 4s..4s+3 whose windows are wA=(4s)//5 and
    wA+1.  m = wA%10 selects the row offset 3m inside the 32-row PSUM block
    b=(wA//10)%4; the matmul writes the aligned prefix [32b, 32b+3m+6) using
    3m leading zero-weight columns.  m==9 entries split in two (the second
    window starts a new 32-block and PSUM column tile).

    Returns a list of dicts: s (rhs index), take (deltas included), gap
    (leading zero cols), nreal (real cols: 3 per window), quad (32b), colblk
    (u50 PSUM column tile), src_off (col in the compact gdt2).
    """
    entries = []
    for s in range(NMM_A):
        wA = (4 * s) // 5
        m = wA % 10
        has_d1 = (4 * s + 3) // 5 > wA
        if m < 9:
            entries.append(dict(
                s=s, take=(0, 1), gap=3 * m, nreal=6,
                quad=32 * ((wA // 10) % 4), colblk=wA // 40, src_off=6 * s))
        else:
            entries.append(dict(
                s=s, take=(0,), gap=27, nreal=3,
                quad=32 * ((wA // 10) % 4), colblk=wA // 40, src_off=6 * s))
            if has_d1:
                w2 = wA + 1
                entries.append(dict(
                    s=s, take=(1,), gap=0, nreal=3,
                    quad=32 * ((w2 // 10) % 4), colblk=w2 // 40,
                    src_off=6 * s + 3))
    # gapped SBUF offsets: zeros [off, off+gap), reals [off+gap, off+gap+nreal)
    off = 0
    for e in entries:
        e["ap_start"] = off
        e["ncols"] = e["gap"] + e["nreal"]
        off += e["ncols"]
    return entries, off


def _precompute(theta):
    """float64 propagator weights, packed into the device operand layouts."""
    theta = np.asarray(theta, np.float64)
    (kSr, kDr, kMr, EaS, EaD, EaM, aSD, aDS, aM, aMSC, uM, cS, cD, cM) = theta
    temp, I_S, I_D = _forcings()
    arr = lambda p, Ea: p * np.exp(-Ea / GAS_R * (1.0 / temp - 1.0 / TEMP_REF))
    k_S, k_D, k_M = arr(kSr, EaS), arr(kDr, EaD), arr(kMr, EaM)

    zeros = np.zeros(N)
    A0 = np.stack([-k_S, aDS * k_D, aM * aMSC * k_M])
    A1 = np.stack([aSD * k_S, -(uM + k_D), aM * (1 - aMSC) * k_M])
    A2 = np.stack([zeros, np.full(N, uM), -k_M])
    W = np.stack([A0, A1, A2]).transpose(2, 0, 1)          # [N,3,3]
    bias = np.stack([I_S, I_D, zeros], axis=1)             # [N,3]

    beta = np.clip(np.array([cS, cD, cM]), 1e-6, None)
    ds = np.sqrt(beta * DT)

    M = np.eye(3)[None] + DT * W[1:]                       # [10000,3,3]
    c = DT * bias[1:]                                      # [10000,3]

    # level A: within-chunk suffix products S10[c,tau] = M_{end}...M_{tau+1}
    Mc = M.reshape(NC1, L1, 3, 3)
    S10 = np.empty((NC1, L1, 3, 3))
    A10 = np.empty((NC1, 3, 3))
    for cI in range(NC1):
        acc = np.eye(3)
        S10[cI, L1 - 1] = acc
        for tau in range(L1 - 2, -1, -1):
            acc = acc @ Mc[cI, tau + 1]
            S10[cI, tau] = acc
        A10[cI] = S10[cI, 0] @ Mc[cI, 0]

    # within-window suffix products over chunks (needed for level C and for
    # folding the window propagation into the level-A weights)
    A10w = A10.reshape(NW, CPW, 3, 3)
    S50 = np.empty((NW, CPW, 3, 3))
    A50 = np.empty((NW, 3, 3))
    for w in range(NW):
        acc = np.eye(3)
        S50[w, CPW - 1] = acc
        for g in range(CPW - 2, -1, -1):
            acc = acc @ A10w[w, g + 1]
            S50[w, g] = acc
        A50[w] = S50[w, 0] @ A10w[w, 0]

    # folded level-A weights: F[c] = S50[w(c),g(c)] @ S10[c,tau] @ diag(ds)
    # so the PE produces U50[w] = sum_{c in w} F[c]^T eps[c] directly.
    # Fmat[c, 3tau+j, i] = sum_k S50c[c,i,k] S10[c,tau,k,j] ds[j]
    S50c = S50.reshape(NC1, 3, 3)
    Fmat = (np.einsum('cik,ctkj->ctij', S50c, S10)
            * ds[None, None, None, :]).transpose(0, 1, 3, 2).reshape(NC1, 30, 3)

    # deterministic trajectory at obs points (exact, float64)
    xd = np.zeros(3)
    detx = np.zeros((NOBS, 3))
    for t in range(NSTEP):
        xd = M[t] @ xd + c[t]
        if (t + 1) % OBS_EVERY == 0:
            detx[(t + 1) // OBS_EVERY] = xd

    # observation weights
    sub = np.arange(NOBS) * OBS_EVERY
    C1 = np.stack([(1 - aSD) * k_S[sub], (1 - aDS) * k_D[sub], (1 - aM) * k_M[sub]],
                  axis=1)
    Wobs = np.concatenate([np.broadcast_to(np.eye(3), (NOBS, 3, 3)),
                           C1[:, None, :]], axis=1)        # [NOBS,4,3]

    # level C: Rmat[(w,j),(n,o)] = sum_i Wobs[n,o,i] PhiW[n,w+1][i,j] (w < n)
    Rmat = np.zeros((3 * NW, NOUT))
    RX = np.zeros((3, NOUT))
    base = np.zeros(NOUT)
    for n in range(NOBS):
        WP = Wobs[n]
        base[4 * n:4 * n + 4] = WP @ detx[n]
        acc = WP.copy()
        for w in range(n - 1, -1, -1):
            Rmat[3 * w:3 * w + 3, 4 * n:4 * n + 4] = acc.T
            acc = acc @ A50[w]
        RX[:, 4 * n:4 * n + 4] = acc.T

    # ---------------- pack into device layouts (fp16) ----------------
    # compact level-A weights gdt2 [128, 6*250]: supergroup s block = 6 cols
    # [window wA comps | window wA+1 comps]; row 32g+r holds chunk 4s+g.
    # The device scatters these into the gapped zero-prefix layout.
    Gd = np.zeros((SUPER, 32, NMM_A, 6), F16)
    for c10 in range(NC1):
        s, g = c10 // SUPER, c10 % SUPER
        d = c10 // CPW - (SUPER * s) // CPW      # 0/1: window rel. to wA
        Gd[g, :30, s, 3 * d:3 * d + 3] = Fmat[c10]
    gdt2 = Gd.reshape(KE, NMM_A * 6)

    # u50 row map: window w, comp j -> row 32*((w//10)%4) + 3*(w%10) + j,
    #                                  col 64*(w//40) + b
    # Rsb: only the nonzero (triangular) column range of each (wt, half)
    # block is shipped; see _rsb_blocks() for the packing.
    blocks = _rsb_blocks()
    ncols = sum(b[3] for b in blocks)
    Rsb = np.zeros((128, ncols), F16)
    for wt, h, rel0, keep, off in blocks:
        blk = np.zeros((128, keep), np.float64)
        for rho in range(128):
            q = rho % 32
            if q >= 30:
                continue
            w = WPS * (4 * wt + rho // 32) + q // 3
            j = q % 3
            blk[rho] = Rmat[3 * w + j, NH * h + rel0:NH * h + rel0 + keep]
        Rsb[:, off:off + keep] = blk

    RXaug = np.concatenate([RX, base[None]], axis=0).astype(F16)  # [4,804]
    return dict(gdt2=gdt2, Rsb=Rsb, RXaug=RXaug)


def _rsb_blocks():
    """Nonzero column ranges of each level-C (wt, half) block.

    Window-tile wt covers windows [40wt, 40wt+40); its rows only affect
    observations n >= 40wt+1, i.e. global cols >= 4*(40wt+1).  Returns
    (wt, h, rel0, keep, packed_col_offset) for each nonempty block.
    """
    blocks = []
    off = 0
    for h in range(2):
        for wt in range(NTILE_B):
            rel0 = max(0, 4 * (40 * wt + 1) - NH * h)
            if rel0 >= NH:
                continue
            keep = NH - rel0
            blocks.append((wt, h, rel0, keep, off))
            off += keep
    return blocks


def _pack_eps(noise_core):
    """[64,10000,3] f16 -> [128, 250*64]: row 32g + (3tau+j), col 64s + b =
    eps[b, t, j] for t = 10*(4s+g) + tau; rows 32g+30, 32g+31 are zero pad."""
    a = noise_core.reshape(BC, NSTEP * 3).T          # [30000, 64] view
    a = np.ascontiguousarray(a).reshape(NSUP_COL, SUPER, 30, BC)
    out = np.zeros((SUPER, 32, NSUP_COL, BC), F16)
    out[:, :30] = a.transpose(1, 2, 0, 3)
    return out.reshape(KE, NSUP_COL * BC)


# ------------------------------------------------------------ bass program
def _build_program(**bass_kwargs):
    import concourse.bass as bass
    import concourse.tile as tile
    from concourse import bacc, mybir

    f32 = mybir.dt.float32
    f16 = mybir.dt.float16
    nc = bacc.Bacc(None, target_bir_lowering=False, **bass_kwargs)

    entries, NGSB = _a_entries()
    PERIOD = 25                       # supergroups per layout period
    PCOLS = NGSB // 10                # gapped cols per period (477)

    eps_d = nc.dram_tensor("eps", [KE, NSUP_COL * BC], f16, kind="ExternalInput")
    gdt_d = nc.dram_tensor("gdt2", [KE, NMM_A * 6], f16, kind="ExternalInput")
    rblocks = _rsb_blocks()
    NRSB = sum(b[3] for b in rblocks)
    rsb_d = nc.dram_tensor("rsb", [128, NRSB], f16, kind="ExternalInput")
    xr_d = nc.dram_tensor("xraug", [4, BC + NOUT], f16, kind="ExternalInput")
    out_d = nc.dram_tensor("out", [BC, NOUT], f16, kind="ExternalOutput")

    # eps slices: the tensor engine chases the DMA stream with one-slice
    # latency; the last slice is small so little work remains after the
    # final byte lands.  Alternating the two HWDGE rings hides per-transfer
    # handoff gaps.
    SLICES = [16, 32, 48, 48, 48, 48, 10]
    SOFF = [0]
    for n_ in SLICES:
        SOFF.append(SOFF[-1] + n_)

    # pipeline triggers, keyed by supergroup index s: u50 PSUM column tile wt
    # is complete after its last writer (s = 50*wt + 49); level-C pieces are
    # deferred ~16 supergroups past their last evac so the PE never stalls on
    # the DVE copy they depend on.
    evac_after = {49: 0, 99: 1, 149: 2, 199: 3, 249: 4}
    chalf_after = {165: 0, 215: 2}    # piece 3 (cols 603:804) runs at the end

    with tile.TileContext(nc) as tc:
        with (
            tc.tile_pool(name="consts", bufs=1) as consts,
            tc.tile_pool(name="epsp", bufs=1) as epsp,
            tc.tile_pool(name="psB", bufs=1, space="PSUM") as psB,
            tc.tile_pool(name="psC", bufs=2, space="PSUM") as psC,
        ):
            gdt = consts.tile([KE, NMM_A * 6], f16)
            gsb = consts.tile([KE, NGSB], f16)       # gapped zero-prefix lhsT
            rsb = consts.tile([128, NRSB], f16)
            xrt = consts.tile([4, BC + NOUT], f16)
            u50 = consts.tile([128, NTILE_B * BC], f16)
            outsb = consts.tile([BC, NOUT], f16)
            x0t = xrt[:, :BC]
            rxt = xrt[:, BC:]

            eps_t = []
            for i, nsup in enumerate(SLICES):
                et = epsp.tile([KE, BC * nsup], f16, tag=f"eps{i}")
                eps_t.append(et)

            # DMA rings: gdt2 first (it gates the weight scatter), then eps
            # slices alternating between the two HWDGE rings; rsb/xr go on
            # the scalar ring behind its eps slices.
            nc.sync.dma_start(out=gdt, in_=gdt_d[:])
            for i in range(len(SLICES)):
                ring = (nc.sync, nc.scalar)[i % 2]
                ring.dma_start(out=eps_t[i],
                               in_=eps_d[:, BC * SOFF[i]:BC * SOFF[i + 1]])
            nc.scalar.dma_start(out=rsb, in_=rsb_d[:])
            nc.scalar.dma_start(out=xrt, in_=xr_d[:])

            # u50 accumulator PSUM: one bank, pre-zeroed (every matmul runs
            # start=False and zero-weight columns only ever add 0.0)
            pu = psB.tile([128, BC * NTILE_B], f32, tag="pu", name="pu")
            nc.vector.memset(pu, 0.0)

            # gapped lhsT build: zero only the gap columns (disjoint from the
            # scattered real columns, so the scatter never waits on the
            # memsets), then copy the compact weights with one strided copy
            # per period-residue entry (the layout is exactly periodic:
            # s -> s+25 shifts the gapped offset by PCOLS, compact col by 150)
            gsb_v = gsb.rearrange("p (k q) -> p k q", q=PCOLS)
            gdt_v = gdt.rearrange("p (k q) -> p k q", q=6 * PERIOD)
            mi = 0
            for e in entries:
                if e["s"] >= PERIOD:
                    break
                if e["gap"]:
                    eng = (nc.gpsimd, nc.vector)[mi % 2]
                    eng.memset(gsb_v[:, :, e["ap_start"]:
                                     e["ap_start"] + e["gap"]], 0.0)
                    mi += 1
                dst = e["ap_start"] + e["gap"]
                nc.vector.tensor_copy(
                    gsb_v[:, :, dst:dst + e["nreal"]],
                    gdt_v[:, :, e["src_off"]:e["src_off"] + e["nreal"]])

            def eps_rhs(s):
                for i in range(len(SLICES)):
                    if s < SOFF[i + 1]:
                        c = s - SOFF[i]
                        return eps_t[i][:, BC * c:BC * (c + 1)]
                raise AssertionError(s)

            def emit_c_piece(lo, hi, ev):
                h = lo // NH
                pc = psC.tile([BC, NH], f32, tag="pc")
                pcv = pc[:, :hi - lo]
                mms = [(x0t, rxt[:, lo:hi], 0, hi - lo)]
                for (wt, bh, rel0, keep, off) in rblocks:
                    if bh != h:
                        continue
                    g0 = max(lo, NH * h + rel0)
                    g1 = min(hi, NH * (h + 1))
                    if g0 < g1:
                        mms.append((u50[:, BC * wt:BC * (wt + 1)],
                                    rsb[:, off + g0 - NH * h - rel0:
                                         off + g1 - NH * h - rel0],
                                    g0 - lo, g1 - lo))
                for mi_, (lhsT, rhs, c0, c1) in enumerate(mms):
                    nc.tensor.matmul(pcv[:, c0:c1], lhsT, rhs,
                                     start=(mi_ == 0),
                                     stop=(mi_ == len(mms) - 1),
                                     skip_group_check=True)
                if ev == 0:
                    nc.vector.tensor_copy(outsb[:, lo:hi], pcv)
                else:
                    nc.scalar.copy(outsb[:, lo:hi], pcv)
                nc.scalar.dma_start(out=out_d[:, lo:hi], in_=outsb[:, lo:hi])

            # ---- fused level-A pipeline: accumulate u50 directly ----
            for e in entries:
                nc.tensor.matmul(
                    pu[e["quad"]:e["quad"] + e["ncols"],
                       BC * e["colblk"]:BC * (e["colblk"] + 1)],
                    gsb[:, e["ap_start"]:e["ap_start"] + e["ncols"]],
                    eps_rhs(e["s"]),
                    start=False, stop=False, tile_position=(0, e["quad"]),
                    skip_group_check=True)
                if e["take"][-1] == 1 or e["s"] % PERIOD == 24 or \
                        e["s"] == NMM_A - 1:
                    s = e["s"]     # last entry of this supergroup
                    if s in evac_after:
                        wt = evac_after[s]
                        nc.vector.tensor_copy(u50[:, BC * wt:BC * (wt + 1)],
                                              pu[:, BC * wt:BC * (wt + 1)])
                    if s in chalf_after:
                        p = chalf_after[s]
                        if p == 0:
                            emit_c_piece(0, NH, 0)
                        else:
                            emit_c_piece(NH, NH + NH // 2 + 1, 1)
            emit_c_piece(NH + NH // 2 + 1, NOUT, 0)

    nc.finalize()
    return nc


# ------------------------------------------------------------------ kernel
def kernel(theta, x0, noise, obs_every):
    global _program_cache, _last_results
    from concourse.bass_utils import run_bass_kernel_spmd

    assert int(obs_every) == OBS_EVERY
    theta = np.asarray(theta, np.float32)
    x0 = np.asarray(x0, np.float32)
    noise = np.asarray(noise, np.float32).astype(F16)

    ops = _precompute(theta.astype(np.float64))

    if _program_cache is None:
        _program_cache = _build_program()
    nc = _program_cache

    in_maps = []
    for q in range(NCORE):
        sl = slice(BC * q, BC * (q + 1))
        x0aug = np.concatenate([np.ascontiguousarray(x0[sl].T),
                                np.ones((1, BC), np.float32)], axis=0).astype(F16)
        in_maps.append({
            "eps": _pack_eps(noise[sl]),
            "gdt2": ops["gdt2"],
            "rsb": ops["Rsb"],
            "xraug": np.concatenate([x0aug, ops["RXaug"]], axis=1),
        })

    import os
    trace = bool(os.environ.get("KERNEL_TRACE"))
    res = run_bass_kernel_spmd(nc, in_maps, core_ids=list(range(NCORE)),
                               trace=trace)
    _last_results = res
    out = np.concatenate(
        [res.results[q]["out"].astype(np.float32).reshape(BC, NOBS, 4)
         for q in range(NCORE)], axis=0)
    return out



# revision 40
# speedup vs baseline: 1.1255x; 1.1255x over previous
"""Trainium2 Bass kernel for the SCON linear-SDE particle scan.

Reference computation: x_{t+1} = (I + DT*W_{t+1}) x_t + DT*b_{t+1} + ds*eps_t
over 10000 steps for B=512 particles with a 3-dim state, observed every 50
steps through a [4,3] projection -> loc_y [512, 201, 4].

The transition matrices depend only on theta (14 scalars), so the whole scan
is a linear map of (x0, eps).  On the host (float64) we precompute propagator
weights that turn the scan into two levels of PE matmuls over the noise:

  level A: each matmul covers 4 chunks of 10 steps; the S50 window suffix-
           products are folded into the weights so the PE accumulates
           window sums U50[w] directly in PSUM (no intermediate U10 level).
           Chunks of the same window sum across K-rows automatically; the
           32-aligned PSUM constraint is met with zero-prefix weight columns.
  level C: obs propagation + projection + x0/deterministic affine part

B is sharded 64 particles per core across 8 cores (pure data parallel).
Per-core device work: stream 3.84 MB of fp16 noise (as lhsT-ready
[128, 250*64] tiles), 260 + 21 matmuls, write [64, 804] fp16 output.
"""

import numpy as np

# ---------------------------------------------------------------- constants
T_TOT = 1000.0
DT = 0.1
N = 10001
TEMP_REF = 283.0
TEMP_RISE = 5.0
GAS_R = 0.008314
NSTEP = N - 1            # 10000
B = 512
NCORE = 8
BC = B // NCORE          # 64 particles per core

L1 = 10                  # level-A chunk length (steps)
NC1 = NSTEP // L1        # 1000 chunks
CPW = 5                  # chunks per window
NW = NC1 // CPW          # 200 windows
NOBS = NW + 1            # 201 observations
OBS_EVERY = 50

SUPER = 4                # chunks per level-A matmul
NMM_A = NC1 // SUPER     # 250 level-A matmuls
KE = 128                 # eps rows per level-A matmul (4 chunks x 32, 2 pad
                         # rows per chunk so blocks start 32-aligned)
NSUP_COL = 250           # eps columns groups (one per matmul)
NTILE_A = (NMM_A + 3) // 4   # 63 psum tiles (4 matmuls/tile, last has 2)

WPS = 10                 # windows per level-B slot (30 rows of 32)
NSLOT_B = NW // WPS      # 20 slots
NTILE_B = NSLOT_B // 4   # 5 u50 tiles
TAUS_PER_SLOT = 4        # u10 tiles touched per level-B slot

NOUT = 4 * NOBS          # 804
NH = NOUT // 2           # 402  (psum free-dim per matmul)

_program_cache = None
_last_results = None     # BassKernelResults of the most recent run (for test.py)

# Validation builds add PSUM memsets so CoreSim's race checker sees no reads
# of never-written rows.  Production skips them: the stale rows only ever
# multiply zero weight columns (and are overwritten data-wise each reuse), so
# they cannot affect results; first-use tiles are still zeroed.
SIM_SAFE = False


# ------------------------------------------------------------- host math
def _forcings():
    times = np.linspace(0.0, T_TOT, N)
    temp = (TEMP_REF + TEMP_RISE * times / (80 * 24 * 365)
            + 10 * np.sin(2 * np.pi / 24 * times)
            + 10 * np.sin(2 * np.pi / (24 * 365) * times))
    I_S = 0.001 + 0.0005 * np.sin(2 * np.pi / (24 * 365) * times)
    I_D = 0.0001 + 5e-05 * np.sin(2 * np.pi / (24 * 365) * times)
    return temp, I_S, I_D


F16 = np.float16


def _a_entries():
    """Level-A matmul entries (theta-independent structure).

    Supergroup s covers chunks 4s..4s+3 whose windows are wA=(4s)//5 and
    wA+1.  m = wA%10 selects the row offset 3m inside the 32-row PSUM block
    b=(wA//10)%4; the matmul writes the aligned prefix [32b, 32b+3m+6) using
    3m leading zero-weight columns.  m==9 entries split in two (the second
    window starts a new 32-block and PSUM column tile).

    Returns a list of dicts: s (rhs index), take (deltas included), gap
    (leading zero cols), nreal (real cols: 3 per window), quad (32b), colblk
    (u50 PSUM column tile), src_off (col in the compact gdt2).
    """
    entries = []
    for s in range(NMM_A):
        wA = (4 * s) // 5
        m = wA % 10
        has_d1 = (4 * s + 3) // 5 > wA
        if m < 9:
            entries.append(dict(
                s=s, take=(0, 1), gap=3 * m, nreal=6,
                quad=32 * ((wA // 10) % 4), colblk=wA // 40, src_off=6 * s))
        else:
            entries.append(dict(
                s=s, take=(0,), gap=27, nreal=3,
                quad=32 * ((wA // 10) % 4), colblk=wA // 40, src_off=6 * s))
            if has_d1:
                w2 = wA + 1
                entries.append(dict(
                    s=s, take=(1,), gap=0, nreal=3,
                    quad=32 * ((w2 // 10) % 4), colblk=w2 // 40,
                    src_off=6 * s + 3))
    # gapped SBUF offsets: zeros [off, off+gap), reals [off+gap, off+gap+nreal)
    off = 0
    for e in entries:
        e["ap_start"] = off
        e["ncols"] = e["gap"] + e["nreal"]
        off += e["ncols"]
    return entries, off


def _precompute(theta):
    """float64 propagator weights, packed into the device operand layouts."""
    theta = np.asarray(theta, np.float64)
    (kSr, kDr, kMr, EaS, EaD, EaM, aSD, aDS, aM, aMSC, uM, cS, cD, cM) = theta
    temp, I_S, I_D = _forcings()
    arr = lambda p, Ea: p * np.exp(-Ea / GAS_R * (1.0 / temp - 1.0 / TEMP_REF))
    k_S, k_D, k_M = arr(kSr, EaS), arr(kDr, EaD), arr(kMr, EaM)

    zeros = np.zeros(N)
    A0 = np.stack([-k_S, aDS * k_D, aM * aMSC * k_M])
    A1 = np.stack([aSD * k_S, -(uM + k_D), aM * (1 - aMSC) * k_M])
    A2 = np.stack([zeros, np.full(N, uM), -k_M])
    W = np.stack([A0, A1, A2]).transpose(2, 0, 1)          # [N,3,3]
    bias = np.stack([I_S, I_D, zeros], axis=1)             # [N,3]

    beta = np.clip(np.array([cS, cD, cM]), 1e-6, None)
    ds = np.sqrt(beta * DT)

    M = np.eye(3)[None] + DT * W[1:]                       # [10000,3,3]
    c = DT * bias[1:]                                      # [10000,3]

    # level A: within-chunk suffix products S10[c,tau] = M_{end}...M_{tau+1}
    Mc = M.reshape(NC1, L1, 3, 3)
    S10 = np.empty((NC1, L1, 3, 3))
    A10 = np.empty((NC1, 3, 3))
    for cI in range(NC1):
        acc = np.eye(3)
        S10[cI, L1 - 1] = acc
        for tau in range(L1 - 2, -1, -1):
            acc = acc @ Mc[cI, tau + 1]
            S10[cI, tau] = acc
        A10[cI] = S10[cI, 0] @ Mc[cI, 0]

    # within-window suffix products over chunks (needed for level C and for
    # folding the window propagation into the level-A weights)
    A10w = A10.reshape(NW, CPW, 3, 3)
    S50 = np.empty((NW, CPW, 3, 3))
    A50 = np.empty((NW, 3, 3))
    for w in range(NW):
        acc = np.eye(3)
        S50[w, CPW - 1] = acc
        for g in range(CPW - 2, -1, -1):
            acc = acc @ A10w[w, g + 1]
            S50[w, g] = acc
        A50[w] = S50[w, 0] @ A10w[w, 0]

    # folded level-A weights: F[c] = S50[w(c),g(c)] @ S10[c,tau] @ diag(ds)
    # so the PE produces U50[w] = sum_{c in w} F[c]^T eps[c] directly.
    # Fmat[c, 3tau+j, i] = sum_k S50c[c,i,k] S10[c,tau,k,j] ds[j]
    S50c = S50.reshape(NC1, 3, 3)
    Fmat = (np.einsum('cik,ctkj->ctij', S50c, S10)
            * ds[None, None, None, :]).transpose(0, 1, 3, 2).reshape(NC1, 30, 3)

    # deterministic trajectory at obs points (exact, float64)
    xd = np.zeros(3)
    detx = np.zeros((NOBS, 3))
    for t in range(NSTEP):
        xd = M[t] @ xd + c[t]
        if (t + 1) % OBS_EVERY == 0:
            detx[(t + 1) // OBS_EVERY] = xd

    # observation weights
    sub = np.arange(NOBS) * OBS_EVERY
    C1 = np.stack([(1 - aSD) * k_S[sub], (1 - aDS) * k_D[sub], (1 - aM) * k_M[sub]],
                  axis=1)
    Wobs = np.concatenate([np.broadcast_to(np.eye(3), (NOBS, 3, 3)),
                           C1[:, None, :]], axis=1)        # [NOBS,4,3]

    # level C: Rmat[(w,j),(n,o)] = sum_i Wobs[n,o,i] PhiW[n,w+1][i,j] (w < n)
    Rmat = np.zeros((3 * NW, NOUT))
    RX = np.zeros((3, NOUT))
    base = np.zeros(NOUT)
    for n in range(NOBS):
        WP = Wobs[n]
        base[4 * n:4 * n + 4] = WP @ detx[n]
        acc = WP.copy()
        for w in range(n - 1, -1, -1):
            Rmat[3 * w:3 * w + 3, 4 * n:4 * n + 4] = acc.T
            acc = acc @ A50[w]
        RX[:, 4 * n:4 * n + 4] = acc.T

    # ---------------- pack into device layouts (fp16) ----------------
    # compact level-A weights gdt2 [128, 6*250]: supergroup s block = 6 cols
    # [window wA comps | window wA+1 comps]; row 32g+r holds chunk 4s+g.
    # The device scatters these into the gapped zero-prefix layout.
    Gd = np.zeros((SUPER, 32, NMM_A, 6), F16)
    for c10 in range(NC1):
        s, g = c10 // SUPER, c10 % SUPER
        d = c10 // CPW - (SUPER * s) // CPW      # 0/1: window rel. to wA
        Gd[g, :30, s, 3 * d:3 * d + 3] = Fmat[c10]
    gdt2 = Gd.reshape(KE, NMM_A * 6)

    # u50 row map: window w, comp j -> row 32*((w//10)%4) + 3*(w%10) + j,
    #                                  col 64*(w//40) + b
    # Rsb: only the nonzero (triangular) column range of each (wt, half)
    # block is shipped; see _rsb_blocks() for the packing.
    blocks = _rsb_blocks()
    ncols = sum(b[3] for b in blocks)
    Rsb = np.zeros((128, ncols), F16)
    for wt, h, rel0, keep, off in blocks:
        blk = np.zeros((128, keep), np.float64)
        for rho in range(128):
            q = rho % 32
            if q >= 30:
                continue
            w = WPS * (4 * wt + rho // 32) + q // 3
            j = q % 3
            blk[rho] = Rmat[3 * w + j, NH * h + rel0:NH * h + rel0 + keep]
        Rsb[:, off:off + keep] = blk

    RXaug = np.concatenate([RX, base[None]], axis=0).astype(F16)  # [4,804]
    return dict(gdt2=gdt2, Rsb=Rsb, RXaug=RXaug)


def _rsb_blocks():
    """Nonzero column ranges of each level-C (wt, half) block.

    Window-tile wt covers windows [40wt, 40wt+40); its rows only affect
    observations n >= 40wt+1, i.e. global cols >= 4*(40wt+1).  Returns
    (wt, h, rel0, keep, packed_col_offset) for each nonempty block.
    """
    blocks = []
    off = 0
    for h in range(2):
        for wt in range(NTILE_B):
            rel0 = max(0, 4 * (40 * wt + 1) - NH * h)
            if rel0 >= NH:
                continue
            keep = NH - rel0
            blocks.append((wt, h, rel0, keep, off))
            off += keep
    return blocks


def _pack_eps(noise_core):
    """[64,10000,3] f16 -> [128, 250*64]: row 32g + (3tau+j), col 64s + b =
    eps[b, t, j] for t = 10*(4s+g) + tau; rows 32g+30, 32g+31 are zero pad."""
    a = noise_core.reshape(BC, NSTEP * 3).T          # [30000, 64] view
    a = np.ascontiguousarray(a).reshape(NSUP_COL, SUPER, 30, BC)
    out = np.zeros((SUPER, 32, NSUP_COL, BC), F16)
    out[:, :30] = a.transpose(1, 2, 0, 3)
    return out.reshape(KE, NSUP_COL * BC)


# ------------------------------------------------------------ bass program
def _build_program(**bass_kwargs):
    import concourse.bass as bass
    import concourse.tile as tile
    from concourse import bacc, mybir

    f32 = mybir.dt.float32
    f16 = mybir.dt.float16
    nc = bacc.Bacc(None, target_bir_lowering=False, **bass_kwargs)

    entries, NGSB = _a_entries()
    PERIOD = 25                       # supergroups per layout period
    PCOLS = NGSB // 10                # gapped cols per period (477)

    eps_d = nc.dram_tensor("eps", [KE, NSUP_COL * BC], f16, kind="ExternalInput")
    gdt_d = nc.dram_tensor("gdt2", [KE, NMM_A * 6], f16, kind="ExternalInput")
    rblocks = _rsb_blocks()
    NRSB = sum(b[3] for b in rblocks)
    rsb_d = nc.dram_tensor("rsb", [128, NRSB], f16, kind="ExternalInput")
    xr_d = nc.dram_tensor("xraug", [4, BC + NOUT], f16, kind="ExternalInput")
    out_d = nc.dram_tensor("out", [BC, NOUT], f16, kind="ExternalOutput")

    # eps slices: the tensor engine chases the DMA stream with one-slice
    # latency; the last slice is small so little work remains after the
    # final byte lands.  Alternating the two HWDGE rings hides per-transfer
    # handoff gaps.
    SLICES = [16, 32, 48, 48, 48, 48, 10]
    SOFF = [0]
    for n_ in SLICES:
        SOFF.append(SOFF[-1] + n_)

    # pipeline triggers, keyed by supergroup index s: u50 PSUM column tile wt
    # is complete after its last writer (s = 50*wt + 49); level-C pieces are
    # deferred ~16 supergroups past their last evac so the PE never stalls on
    # the DVE copy they depend on.
    evac_after = {49: 0, 99: 1, 149: 2, 199: 3, 249: 4}
    chalf_after = {165: 0, 215: 2}    # piece 3 (cols 603:804) runs at the end

    with tile.TileContext(nc) as tc:
        with (
            tc.tile_pool(name="consts", bufs=1) as consts,
            tc.tile_pool(name="epsp", bufs=1) as epsp,
            tc.tile_pool(name="psB", bufs=1, space="PSUM") as psB,
            tc.tile_pool(name="psC", bufs=2, space="PSUM") as psC,
        ):
            gdt = consts.tile([KE, NMM_A * 6], f16)
            gsb = consts.tile([KE, NGSB], f16)       # gapped zero-prefix lhsT
            rsb = consts.tile([128, NRSB], f16)
            xrt = consts.tile([4, BC + NOUT], f16)
            u50 = consts.tile([128, NTILE_B * BC], f16)
            outsb = consts.tile([BC, NOUT], f16)
            x0t = xrt[:, :BC]
            rxt = xrt[:, BC:]

            eps_t = []
            for i, nsup in enumerate(SLICES):
                et = epsp.tile([KE, BC * nsup], f16, tag=f"eps{i}")
                eps_t.append(et)

            # DMA rings: gdt2 first (it gates the weight scatter), then eps
            # slices alternating between the two HWDGE rings; rsb/xr go on
            # the scalar ring behind its eps slices.
            nc.sync.dma_start(out=gdt, in_=gdt_d[:])
            for i in range(len(SLICES)):
                nc.sync.dma_start(out=eps_t[i],
                                  in_=eps_d[:, BC * SOFF[i]:BC * SOFF[i + 1]])
            nc.scalar.dma_start(out=rsb, in_=rsb_d[:])
            nc.scalar.dma_start(out=xrt, in_=xr_d[:])

            # u50 accumulator PSUM: one bank, pre-zeroed (every matmul runs
            # start=False and zero-weight columns only ever add 0.0)
            pu = psB.tile([128, BC * NTILE_B], f32, tag="pu", name="pu")
            nc.vector.memset(pu, 0.0)

            # gapped lhsT build: zero only the gap columns (disjoint from the
            # scattered real columns, so the scatter never waits on the
            # memsets), then copy the compact weights with one strided copy
            # per period-residue entry (the layout is exactly periodic:
            # s -> s+25 shifts the gapped offset by PCOLS, compact col by 150)
            gsb_v = gsb.rearrange("p (k q) -> p k q", q=PCOLS)
            gdt_v = gdt.rearrange("p (k q) -> p k q", q=6 * PERIOD)
            mi = 0
            for e in entries:
                if e["s"] >= PERIOD:
                    break
                if e["gap"]:
                    eng = (nc.gpsimd, nc.vector)[mi % 2]
                    eng.memset(gsb_v[:, :, e["ap_start"]:
                                     e["ap_start"] + e["gap"]], 0.0)
                    mi += 1
                dst = e["ap_start"] + e["gap"]
                nc.vector.tensor_copy(
                    gsb_v[:, :, dst:dst + e["nreal"]],
                    gdt_v[:, :, e["src_off"]:e["src_off"] + e["nreal"]])

            def eps_rhs(s):
                for i in range(len(SLICES)):
                    if s < SOFF[i + 1]:
                        c = s - SOFF[i]
                        return eps_t[i][:, BC * c:BC * (c + 1)]
                raise AssertionError(s)

            def emit_c_piece(lo, hi, ev):
                h = lo // NH
                pc = psC.tile([BC, NH], f32, tag="pc")
                pcv = pc[:, :hi - lo]
                mms = [(x0t, rxt[:, lo:hi], 0, hi - lo)]
                for (wt, bh, rel0, keep, off) in rblocks:
                    if bh != h:
                        continue
                    g0 = max(lo, NH * h + rel0)
                    g1 = min(hi, NH * (h + 1))
                    if g0 < g1:
                        mms.append((u50[:, BC * wt:BC * (wt + 1)],
                                    rsb[:, off + g0 - NH * h - rel0:
                                         off + g1 - NH * h - rel0],
                                    g0 - lo, g1 - lo))
                for mi_, (lhsT, rhs, c0, c1) in enumerate(mms):
                    nc.tensor.matmul(pcv[:, c0:c1], lhsT, rhs,
                                     start=(mi_ == 0),
                                     stop=(mi_ == len(mms) - 1),
                                     skip_group_check=True)
                if ev == 0:
                    nc.vector.tensor_copy(outsb[:, lo:hi], pcv)
                else:
                    nc.scalar.copy(outsb[:, lo:hi], pcv)
                nc.scalar.dma_start(out=out_d[:, lo:hi], in_=outsb[:, lo:hi])

            # ---- fused level-A pipeline: accumulate u50 directly ----
            for e in entries:
                nc.tensor.matmul(
                    pu[e["quad"]:e["quad"] + e["ncols"],
                       BC * e["colblk"]:BC * (e["colblk"] + 1)],
                    gsb[:, e["ap_start"]:e["ap_start"] + e["ncols"]],
                    eps_rhs(e["s"]),
                    start=False, stop=False, tile_position=(0, e["quad"]),
                    skip_group_check=True)
                if e["take"][-1] == 1 or e["s"] % PERIOD == 24 or \
                        e["s"] == NMM_A - 1:
                    s = e["s"]     # last entry of this supergroup
                    if s in evac_after:
                        wt = evac_after[s]
                        nc.vector.tensor_copy(u50[:, BC * wt:BC * (wt + 1)],
                                              pu[:, BC * wt:BC * (wt + 1)])
                    if s in chalf_after:
                        p = chalf_after[s]
                        if p == 0:
                            emit_c_piece(0, NH, 0)
                        else:
                            emit_c_piece(NH, NH + NH // 2 + 1, 1)
            emit_c_piece(NH + NH // 2 + 1, NOUT, 0)

    nc.finalize()
    return nc


# ------------------------------------------------------------------ kernel
def kernel(theta, x0, noise, obs_every):
    global _program_cache, _last_results
    from concourse.bass_utils import run_bass_kernel_spmd

    assert int(obs_every) == OBS_EVERY
    theta = np.asarray(theta, np.float32)
    x0 = np.asarray(x0, np.float32)
    noise = np.asarray(noise, np.float32).astype(F16)

    ops = _precompute(theta.astype(np.float64))

    if _program_cache is None:
        _program_cache = _build_program()
    nc = _program_cache

    in_maps = []
    for q in range(NCORE):
        sl = slice(BC * q, BC * (q + 1))
        x0aug = np.concatenate([np.ascontiguousarray(x0[sl].T),
                                np.ones((1, BC), np.float32)], axis=0).astype(F16)
        in_maps.append({
            "eps": _pack_eps(noise[sl]),
            "gdt2": ops["gdt2"],
            "rsb": ops["Rsb"],
            "xraug": np.concatenate([x0aug, ops["RXaug"]], axis=1),
        })

    import os
    trace = bool(os.environ.get("KERNEL_TRACE"))
    res = run_bass_kernel_spmd(nc, in_maps, core_ids=list(range(NCORE)),
                               trace=trace)
    _last_results = res
    out = np.concatenate(
        [res.results[q]["out"].astype(np.float32).reshape(BC, NOBS, 4)
         for q in range(NCORE)], axis=0)
    return out

